# revision 32
# baseline (speedup 1.0000x reference)
"""Trainium2 Bass kernel for nn_Attention (B=16, N=1024, C=1024, H=16, pre-LN +
q/k post-LN attention block), data-parallel over 8 NeuronCores (2 batches/core).

Per core (batch shard [2, 1024, 1024]), software-pipelined across the two
batches with persistent SBUF buffers and bf16 activations/weights (fp32 PSUM
accumulation and LN statistics):
  A1: y = LN(x) over C (bn_stats fp32); y cast bf16; 8 PE bf16-transposes per
      token tile packed into ONE PSUM bank, evacuated with a single DVE copy.
  A2: qT/kT = W'c @ yT (weights host-pre-centered per head so the post-LN mean
      subtraction folds in; the 1/8 attention scale folds into q's rstd eps);
      per-head rstd via ACT Square + ones-matmul partition reduction + ACT
      sqrt + DVE bf16 reciprocal, broadcast back via a PE matmul; v in
      [tok, d] layout with a ones column (softmax denominator for free).
  B:  per head-pair / query-chunk: S^T tiles for both heads land in one
      2-bank PSUM window, ONE exp [128,1024] on ScalarE, O_aug^T = [V|1]^T E
      accumulated on PE (row 64 = denominator); normalize via DVE reciprocal
      + per-head PE broadcast + DVE muls straight out of PSUM (head1 shifted
      into partitions 64-127 by a small SBUF-SBUF DMA).
  C:  out = AO^T^T @ Wp^T; bias added during the PSUM evacuation (DVE
      tensor add against a host-broadcast bias tile), fp32 out.

Weights are DMA'd once and stay resident in SBUF (bf16, 8 MB). Phases of
adjacent batches overlap: A2(b) with C(b-1), B(b) with A1(b+1), which keeps
PE dense while ScalarE chews through the exp windows.
"""

import numpy as np

B, N, C, H, Dh = 16, 1024, 1024, 16, 64
NCORES = 8
BL = B // NCORES          # batches per core
T = BL * N                # tokens per core
CCH = C // 128            # contraction chunks
NB = N // 128             # token tiles per batch
EPS = 1e-6

_cache: dict = {}


def _build():
    from contextlib import ExitStack

    import concourse.bacc as bacc
    import concourse.mybir as mybir
    import concourse.tile as tile

    F32 = mybir.dt.float32
    BF16 = mybir.dt.bfloat16
    AF = mybir.ActivationFunctionType
    OP = mybir.AluOpType

    nc = bacc.Bacc("TRN2", target_bir_lowering=False, debug=False,
                   num_devices=NCORES)

    x_d = nc.dram_tensor("x", [T, C], F32, kind="ExternalInput").ap()
    wqt_d = nc.dram_tensor("wqt", [C, C], BF16, kind="ExternalInput").ap()
    wkt_d = nc.dram_tensor("wkt", [C, C], BF16, kind="ExternalInput").ap()
    wvt_d = nc.dram_tensor("wvt", [C, C], BF16, kind="ExternalInput").ap()
    wpt_d = nc.dram_tensor("wpt", [C, C], BF16, kind="ExternalInput").ap()
    bpb_d = nc.dram_tensor("bpb", [128, C], BF16, kind="ExternalInput").ap()
    ce2_d = nc.dram_tensor("c_e2", [128, 2], BF16, kind="ExternalInput").ap()
    cident_d = nc.dram_tensor("c_ident", [128, 128], BF16,
                              kind="ExternalInput").ap()
    cones_d = nc.dram_tensor("c_ones", [128, 64], BF16,
                             kind="ExternalInput").ap()
    cb2_d = nc.dram_tensor("c_b2", [2, 128], BF16, kind="ExternalInput").ap()
    ceps_d = nc.dram_tensor("c_eps", [128, 2], F32, kind="ExternalInput").ap()
    out_d = nc.dram_tensor("out", [T, C], F32, kind="ExternalOutput").ap()

    with tile.TileContext(nc) as tc, ExitStack() as top:
        # ---- persistent SBUF ----
        const = top.enter_context(tc.tile_pool(name="const", bufs=1))
        ident = const.tile([128, 128], BF16)
        nc.sync.dma_start(out=ident, in_=cident_d)
        e2 = const.tile([128, 2], BF16)
        nc.sync.dma_start(out=e2, in_=ce2_d)
        b2 = const.tile([2, 128], BF16)
        nc.sync.dma_start(out=b2, in_=cb2_d)
        cones = const.tile([128, 64], BF16)
        nc.sync.dma_start(out=cones, in_=cones_d)
        ceps = const.tile([128, 2], F32)
        nc.sync.dma_start(out=ceps, in_=ceps_d)
        eps_t = ceps[:, 0:1]
        eps64_t = ceps[:, 1:2]
        bpb = const.tile([128, C], BF16)

        wpool = top.enter_context(tc.tile_pool(name="w", bufs=1))
        w_sb = {}
        w_dram = {"q": wqt_d, "k": wkt_d, "v": wvt_d, "p": wpt_d}
        for nm in ("q", "k", "v", "p"):
            w_sb[nm] = wpool.tile([128, CCH, C], BF16, name=f"w{nm}")

        def load_weight(nm):
            nc.sync.dma_start(
                out=w_sb[nm],
                in_=w_dram[nm].rearrange("(cc p) d -> p cc d", p=128))

        big = top.enter_context(tc.tile_pool(name="big", bufs=1))
        yT = big.tile([128, CCH, N], BF16, name="yT")
        qT = big.tile([128, CCH, N], BF16, name="qT")
        kT = big.tile([128, CCH, N], BF16, name="kT")
        vS = big.tile([128, NB, H, Dh + 1], BF16, name="vS")
        AO = [big.tile([128, CCH, N], BF16, name=f"AO{i}") for i in range(2)]

        # A1 SBUF scratch (persistent pools, rotating bufs)
        xp = top.enter_context(tc.tile_pool(name="xp", bufs=2))
        yp = top.enter_context(tc.tile_pool(name="yp", bufs=4))
        sp = top.enter_context(tc.tile_pool(name="sp", bufs=3))

        def A1(b, tpp, tag="ps"):
            for t in range(NB):
                r0 = b * N + t * 128
                xt = xp.tile([128, C], F32, tag="xt")
                nc.sync.dma_start(out=xt, in_=x_d[r0:r0 + 128, :])
                stats = sp.tile([128, 2, nc.vector.BN_STATS_DIM], F32,
                                tag="st")
                xg = xt.rearrange("p (s f) -> p s f", s=2)
                for s in range(2):
                    nc.vector.bn_stats(out=stats[:, s, :], in_=xg[:, s, :])
                mv = sp.tile([128, nc.vector.BN_AGGR_DIM], F32, tag="mv")
                nc.vector.bn_aggr(out=mv, in_=stats)
                std = sp.tile([128, 1], F32, tag="sd")
                nc.scalar.activation(std, mv[:, 1:2], AF.Sqrt, bias=eps_t)
                rstd = sp.tile([128, 1], F32, tag="rs")
                nc.vector.reciprocal(rstd, std)
                y = yp.tile([128, C], BF16, tag="y")
                nc.vector.tensor_scalar(
                    out=y, in0=xt, scalar1=mv[:, 0:1], scalar2=rstd,
                    op0=OP.subtract, op1=OP.mult)
                tpb = tpp.tile([128, CCH, 128], BF16, tag=tag, name="tpb")
                for cc in range(CCH):
                    nc.tensor.transpose(
                        tpb[:, cc, :], y[:, cc * 128:(cc + 1) * 128], ident)
                nc.vector.tensor_copy(
                    out=yT[:, :, t * 128:(t + 1) * 128], in_=tpb)

        # A2 scratch
        a2s = top.enter_context(tc.tile_pool(name="a2s", bufs=2))
        # B scratch
        ep = top.enter_context(tc.tile_pool(name="ep", bufs=3))
        rp = top.enter_context(tc.tile_pool(name="rp", bufs=1))
        bcsp = top.enter_context(tc.tile_pool(name="bcsp", bufs=2))
        tb = top.enter_context(tc.tile_pool(name="tb", bufs=2))
        # C scratch
        op_ = top.enter_context(tc.tile_pool(name="op", bufs=2))

        def A2(b, pp, sqp, bcp):
            for wi, (wn, o_big) in enumerate((("q", qT), ("k", kT))):
                wsb = w_sb[wn]
                for dc in range(CCH):
                    for t2 in range(2):
                        ps = pp.tile([128, 512], F32, tag="ps")
                        for cc in range(CCH):
                            nc.tensor.matmul(
                                ps, wsb[:, cc, dc * 128:(dc + 1) * 128],
                                yT[:, cc, t2 * 512:(t2 + 1) * 512],
                                start=(cc == 0), stop=(cc == CCH - 1))
                        sq = a2s.tile([128, 512], BF16, tag="sq")
                        nc.scalar.activation(sq, ps, AF.Square)
                        qraw = a2s.tile([128, 512], BF16, tag="qraw")
                        nc.vector.tensor_copy(out=qraw, in_=ps)
                        ssq = sqp.tile([2, 512], F32, tag="ssq")
                        nc.tensor.matmul(ssq, e2, sq, start=True, stop=True)
                        stdt = a2s.tile([2, 512], BF16, tag="stdt")
                        if wi == 0:
                            # 0.125/sqrt(ssq/64+eps) = 1/sqrt(ssq+64eps)
                            nc.scalar.activation(
                                stdt, ssq, AF.Sqrt, bias=eps64_t[0:2, :])
                        else:
                            nc.scalar.activation(
                                stdt, ssq, AF.Sqrt, bias=eps_t[0:2, :],
                                scale=1.0 / 64.0)
                        rst = a2s.tile([2, 512], BF16, tag="rst")
                        with nc.allow_low_precision(reason="bf16 rstd"):
                            nc.vector.reciprocal(rst, stdt)
                        bc = bcp.tile([128, 512], F32, tag="bc")
                        nc.tensor.matmul(bc, b2, rst, start=True, stop=True)
                        nc.vector.tensor_mul(
                            o_big[:, dc, t2 * 512:(t2 + 1) * 512], qraw, bc)
            # v projection + ones column
            wsb = w_sb["v"]
            for tt in range(NB):
                for d2 in range(2):
                    ps = pp.tile([128, 512], F32, tag="ps")
                    for cc in range(CCH):
                        nc.tensor.matmul(
                            ps, yT[:, cc, tt * 128:(tt + 1) * 128],
                            wsb[:, cc, d2 * 512:(d2 + 1) * 512],
                            start=(cc == 0), stop=(cc == CCH - 1))
                    nc.vector.tensor_copy(
                        out=vS[:, tt, d2 * 8:(d2 + 1) * 8, 0:64],
                        in_=ps.rearrange("p (h e) -> p h e", e=64))
                nc.vector.tensor_copy(
                    out=vS[:, tt, :, 64:65],
                    in_=cones[:, 0:H].rearrange("p (h e) -> p h e", e=1))

        def Bphase(b, swin, oop, nbc, cpp, cunits, first):
            AOc = AO[b % 2]
            def emit_S(hp, qc):
                sw = swin.tile([128, 1024], F32, tag="sw", name="sw")
                nc.tensor.matmul(
                    sw[:, 0:512],
                    kT[0:64, hp, emit_S.kc * 128:(emit_S.kc + 1) * 128],
                    qT[0:64, hp, qc * 512:(qc + 1) * 512],
                    start=True, stop=True)
                nc.tensor.matmul(
                    sw[:, 512:1024],
                    kT[64:128, hp, emit_S.kc * 128:(emit_S.kc + 1) * 128],
                    qT[64:128, hp, qc * 512:(qc + 1) * 512],
                    start=True, stop=True)
                return sw

            for qc in range(2):
                for hp in range(H // 2):
                    cu = cunits.pop(0) if (qc == 1 or not first) and cunits \
                        else None
                    if cu is not None:
                        cu()
                    oo = oop.tile([65, 2, 512], F32, tag="oo")
                    # software-pipelined by one kc stage: S(kc+1) is emitted
                    # before exp(kc)/O(kc) so PE fills the exp window and
                    # ScalarE never starves.
                    emit_S.kc = 0
                    sw_prev = emit_S(hp, qc)
                    for kc in range(NB):
                        if kc + 1 < NB:
                            emit_S.kc = kc + 1
                            sw_next = emit_S(hp, qc)
                        ew = ep.tile([128, 1024], BF16, tag="ew")
                        nc.scalar.activation(ew, sw_prev, AF.Exp)
                        nc.tensor.matmul(
                            oo[:, 0, :], vS[:, kc, 2 * hp, :], ew[:, 0:512],
                            start=(kc == 0), stop=(kc == NB - 1))
                        nc.tensor.matmul(
                            oo[:, 1, :], vS[:, kc, 2 * hp + 1, :],
                            ew[:, 512:1024],
                            start=(kc == 0), stop=(kc == NB - 1))
                        if kc + 1 < NB:
                            sw_prev = sw_next
                    r2 = rp.tile([128, 2, 512], BF16, tag="r2")
                    with nc.allow_low_precision(reason="bf16 recip"):
                        nc.vector.reciprocal(r2[64:65, :, :], oo[64:65, :, :])
                    osb = bcsp.tile([64, 2, 512], BF16, tag="osb")
                    nc.vector.tensor_copy(out=osb, in_=oo[0:64, :, :])
                    bc0 = nbc.tile([64, 512], F32, tag="nbc")
                    nc.tensor.matmul(
                        bc0, cones[64:65, :], r2[64:65, 0, :],
                        start=True, stop=True, tile_position=(64, 0))
                    bc1 = nbc.tile([64, 512], F32, tag="nbc")
                    nc.tensor.matmul(
                        bc1, cones[64:65, :], r2[64:65, 1, :],
                        start=True, stop=True, tile_position=(64, 0))
                    nc.vector.tensor_mul(
                        AOc[0:64, hp, qc * 512:(qc + 1) * 512],
                        osb[:, 0, :], bc0)
                    t2b = tb.tile([64, 512], BF16, tag="t2b")
                    nc.vector.tensor_mul(t2b, osb[:, 1, :], bc1)
                    nc.sync.dma_start(
                        out=AOc[64:128, hp, qc * 512:(qc + 1) * 512], in_=t2b)

        def C_unit(b, pp, tt, d2, tag="ps"):
            AOc = AO[b % 2]
            wsb = w_sb["p"]
            ps = pp.tile([128, 512], F32, tag=tag, name="cps")
            for cc in range(CCH):
                nc.tensor.matmul(
                    ps, AOc[:, cc, tt * 128:(tt + 1) * 128],
                    wsb[:, cc, d2 * 512:(d2 + 1) * 512],
                    start=(cc == 0), stop=(cc == CCH - 1))
            o_sb = op_.tile([128, 512], F32, tag="osb")
            nc.vector.tensor_add(
                o_sb, ps, bpb[:, d2 * 512:(d2 + 1) * 512])
            nc.sync.dma_start(
                out=out_d[b * N + tt * 128:b * N + (tt + 1) * 128,
                          d2 * 512:(d2 + 1) * 512],
                in_=o_sb)

        # ---- pipelined schedule ----
        # A1(b)'s LN work (DMA + DVE) has no PSUM deps, so it executes during
        # B(b-1); only its transposes wait for the scope1 banks.
        for b in range(BL):
            with ExitStack() as ph:
                pp = ph.enter_context(
                    tc.tile_pool(name="pp", bufs=4, space="PSUM"))
                sqp = ph.enter_context(
                    tc.tile_pool(name="sqp", bufs=2, space="PSUM"))
                bcp = ph.enter_context(
                    tc.tile_pool(name="bcp", bufs=2, space="PSUM"))
                A1(b, pp)
                if b == 0:
                    for nm in ("q", "k", "v", "p"):
                        load_weight(nm)
                    nc.sync.dma_start(out=bpb, in_=bpb_d)
                A2(b, pp, sqp, bcp)
            with ExitStack() as ph:
                swin = ph.enter_context(
                    tc.tile_pool(name="swin", bufs=2, space="PSUM"))
                oop = ph.enter_context(
                    tc.tile_pool(name="oop", bufs=1, space="PSUM"))
                nbc = ph.enter_context(
                    tc.tile_pool(name="nbc", bufs=1, space="PSUM"))
                cpp = ph.enter_context(
                    tc.tile_pool(name="cpp", bufs=1, space="PSUM"))
                # C units of the previous batch (all ready) and of this
                # batch's qc=0 half (ready once qc=0 completes) fill the PE
                # idle inside the ACT-bound B window.
                cunits = []
                if b > 0:
                    cunits += [
                        (lambda tt=tt, d2=d2: C_unit(b - 1, cpp, tt, d2,
                                                     tag="cps"))
                        for tt in range(4, NB) for d2 in range(2)]
                cunits += [
                    (lambda tt=tt, d2=d2: C_unit(b, cpp, tt, d2, tag="cps"))
                    for tt in range(0, 4) for d2 in range(2)]
                Bphase(b, swin, oop, nbc, cpp, cunits, first=(b == 0))
        with ExitStack() as ph:
            ppc = ph.enter_context(
                tc.tile_pool(name="ppc", bufs=3, space="PSUM"))
            for tt in range(4, NB):
                for d2 in range(2):
                    C_unit(BL - 1, ppc, tt, d2)

    nc.compile()
    return nc


def _get_nc():
    if "nc" not in _cache:
        _cache["nc"] = _build()
    return _cache["nc"]


def _host_inputs(Wq, Wk, Wv, Wp, bp):
    """Shared (core-independent) derived weight tensors."""
    import ml_dtypes
    BF = ml_dtypes.bfloat16

    def center(Wm):
        Wh = np.asarray(Wm, np.float32).reshape(H, Dh, C)
        return (Wh - Wh.mean(axis=1, keepdims=True)).reshape(C, C)

    e2 = np.zeros((128, 2), np.float32)
    e2[0:64, 0] = 1.0
    e2[64:128, 1] = 1.0
    b2 = np.zeros((2, 128), np.float32)
    b2[0, 0:64] = 1.0
    b2[1, 64:128] = 1.0
    eps = np.zeros((128, 2), np.float32)
    eps[:, 0] = EPS
    eps[:, 1] = 64.0 * EPS
    bpb = np.broadcast_to(np.asarray(bp, np.float32).reshape(1, C),
                          (128, C))
    return {
        "c_e2": e2.astype(BF),
        "c_b2": b2.astype(BF),
        "c_ones": np.ones((128, 64), BF),
        "c_ident": np.eye(128, dtype=np.float32).astype(BF),
        "c_eps": eps,
        "bpb": np.ascontiguousarray(bpb).astype(BF),
        "wqt": np.ascontiguousarray(center(Wq).T).astype(BF),
        "wkt": np.ascontiguousarray(center(Wk).T).astype(BF),
        "wvt": np.ascontiguousarray(np.asarray(Wv, np.float32).T).astype(BF),
        "wpt": np.ascontiguousarray(np.asarray(Wp, np.float32).T).astype(BF),
    }


def kernel(x, Wq, Wk, Wv, Wp, bp):
    from concourse.bass_utils import run_bass_kernel_spmd

    nc = _get_nc()
    shared = _host_inputs(Wq, Wk, Wv, Wp, bp)
    x = np.asarray(x, dtype=np.float32)
    in_maps = [
        dict(shared, x=np.ascontiguousarray(x[c * BL:(c + 1) * BL].reshape(T, C)))
        for c in range(NCORES)
    ]
    res = run_bass_kernel_spmd(nc, in_maps, core_ids=list(range(NCORES)))
    out = np.stack([res.results[c]["out"].reshape(BL, N, C)
                    for c in range(NCORES)])
    return out.reshape(B, N, C).astype(np.float32)


# revision 33
# speedup vs baseline: 1.1670x; 1.1670x over previous
"""Trainium2 Bass kernel for nn_Attention (B=16, N=1024, C=1024, H=16, pre-LN +
q/k post-LN attention block), data-parallel over 8 NeuronCores (2 batches/core).

Per core (batch shard [2, 1024, 1024]), software-pipelined across the two
batches with persistent SBUF buffers and bf16 activations/weights (fp32 PSUM
accumulation and LN statistics):
  A1: y = LN(x) over C (bn_stats fp32); y cast bf16; 8 PE bf16-transposes per
      token tile packed into ONE PSUM bank, evacuated with a single DVE copy.
  A2: qT/kT = W'c @ yT (weights host-pre-centered per head so the post-LN mean
      subtraction folds in; the 1/8 attention scale folds into q's rstd eps);
      per-head rstd via ACT Square + ones-matmul partition reduction + ACT
      sqrt + DVE bf16 reciprocal, broadcast back across partitions with a PE
      matmul; q_raw staged to SBUF so the normalize mul reads only one PSUM
      operand (walrus allows at most one PSUM input per DVE op); v in
      [tok, d] layout with a ones column (softmax denominator for free).
  B:  per head-pair / query-chunk: S^T tiles for both heads land in one
      2-bank PSUM window, ONE exp [128,1024] on ScalarE (the gate of this
      phase; ~92% ACT occupancy), O_aug^T = [V|1]^T E accumulated on PE
      (row 64 = denominator); normalize via one wide DVE reciprocal + one
      wide PSUM->SBUF staging copy + per-head PE broadcasts + DVE muls
      (head1 shifted into partitions 64-127 by a small SBUF-SBUF DMA).
  C:  out = AO^T^T @ Wp^T; bias added during the PSUM evacuation (DVE tensor
      add against a host-broadcast bias tile), fp32 out. C units are
      interleaved INTO the ACT-bound B windows (C(b-1) second half and C(b)
      first half), so only half of the last batch's projection runs serially
      at the end.

Weights are DMA'd once and stay resident in SBUF (bf16, 8 MB). PSUM is the
scarce resource (8 banks): scope1 = {4x matmul/transpose staging, 2x ssq,
2x rstd-broadcast}, scope2 = {2x2 exp windows, 2 O-accumulators, 1 bc,
1 interleaved-C}. Engine budget per core (cost model): PE ~496us busy,
ACT ~349us, DVE ~324us, span ~603us.

Note: gpsimd.partition_broadcast is NOT used — on real TRN2 it only works
for a physical-partition-0 source and partition-0-based destination
(sim/CoreSim does not model this; verified by hardware micro-test).
"""

import numpy as np

B, N, C, H, Dh = 16, 1024, 1024, 16, 64
NCORES = 8
BL = B // NCORES          # batches per core
T = BL * N                # tokens per core
CCH = C // 128            # contraction chunks
NB = N // 128             # token tiles per batch
EPS = 1e-6

_cache: dict = {}


def _build():
    from contextlib import ExitStack

    import concourse.bacc as bacc
    import concourse.mybir as mybir
    import concourse.tile as tile

    F32 = mybir.dt.float32
    BF16 = mybir.dt.bfloat16
    AF = mybir.ActivationFunctionType
    OP = mybir.AluOpType

    nc = bacc.Bacc("TRN2", target_bir_lowering=False, debug=False,
                   num_devices=NCORES)

    x_d = nc.dram_tensor("x", [T, C], F32, kind="ExternalInput").ap()
    wqt_d = nc.dram_tensor("wqt", [C, C], BF16, kind="ExternalInput").ap()
    wkt_d = nc.dram_tensor("wkt", [C, C], BF16, kind="ExternalInput").ap()
    wvt_d = nc.dram_tensor("wvt", [C, C], BF16, kind="ExternalInput").ap()
    wpt_d = nc.dram_tensor("wpt", [C, C], BF16, kind="ExternalInput").ap()
    bpb_d = nc.dram_tensor("bpb", [128, C], BF16, kind="ExternalInput").ap()
    ce2_d = nc.dram_tensor("c_e2", [128, 2], BF16, kind="ExternalInput").ap()
    cident_d = nc.dram_tensor("c_ident", [128, 128], BF16,
                              kind="ExternalInput").ap()
    cones_d = nc.dram_tensor("c_ones", [128, 64], BF16,
                             kind="ExternalInput").ap()
    cb2_d = nc.dram_tensor("c_b2", [2, 128], BF16, kind="ExternalInput").ap()
    ceps_d = nc.dram_tensor("c_eps", [128, 2], F32, kind="ExternalInput").ap()
    out_d = nc.dram_tensor("out", [T, C], F32, kind="ExternalOutput").ap()

    with tile.TileContext(nc) as tc, ExitStack() as top:
        # ---- persistent SBUF ----
        const = top.enter_context(tc.tile_pool(name="const", bufs=1))
        ident = const.tile([128, 128], BF16)
        nc.sync.dma_start(out=ident, in_=cident_d)
        e2 = const.tile([128, 2], BF16)
        nc.sync.dma_start(out=e2, in_=ce2_d)
        b2 = const.tile([2, 128], BF16)
        nc.sync.dma_start(out=b2, in_=cb2_d)
        cones = const.tile([128, 64], BF16)
        nc.sync.dma_start(out=cones, in_=cones_d)
        ceps = const.tile([128, 2], F32)
        nc.sync.dma_start(out=ceps, in_=ceps_d)
        eps_t = ceps[:, 0:1]
        eps64_t = ceps[:, 1:2]
        bpb = const.tile([128, C], BF16)

        wpool = top.enter_context(tc.tile_pool(name="w", bufs=1))
        w_sb = {}
        w_dram = {"q": wqt_d, "k": wkt_d, "v": wvt_d, "p": wpt_d}
        for nm in ("q", "k", "v", "p"):
            w_sb[nm] = wpool.tile([128, CCH, C], BF16, name=f"w{nm}")

        def load_weight(nm):
            nc.sync.dma_start(
                out=w_sb[nm],
                in_=w_dram[nm].rearrange("(cc p) d -> p cc d", p=128))

        big = top.enter_context(tc.tile_pool(name="big", bufs=1))
        yT = big.tile([128, CCH, N], BF16, name="yT")
        qT = big.tile([128, CCH, N], BF16, name="qT")
        kT = big.tile([128, CCH, N], BF16, name="kT")
        vS = big.tile([128, NB, H, Dh + 1], BF16, name="vS")
        AO = [big.tile([128, CCH, N], BF16, name=f"AO{i}") for i in range(2)]

        # A1 SBUF scratch (persistent pools, rotating bufs)
        xp = top.enter_context(tc.tile_pool(name="xp", bufs=2))
        yp = top.enter_context(tc.tile_pool(name="yp", bufs=4))
        sp = top.enter_context(tc.tile_pool(name="sp", bufs=3))

        def A1(b, tpp, tag="ps"):
            for t in range(NB):
                r0 = b * N + t * 128
                xt = xp.tile([128, C], F32, tag="xt")
                nc.sync.dma_start(out=xt, in_=x_d[r0:r0 + 128, :])
                stats = sp.tile([128, 2, nc.vector.BN_STATS_DIM], F32,
                                tag="st")
                xg = xt.rearrange("p (s f) -> p s f", s=2)
                for s in range(2):
                    nc.vector.bn_stats(out=stats[:, s, :], in_=xg[:, s, :])
                mv = sp.tile([128, nc.vector.BN_AGGR_DIM], F32, tag="mv")
                nc.vector.bn_aggr(out=mv, in_=stats)
                std = sp.tile([128, 1], F32, tag="sd")
                nc.scalar.activation(std, mv[:, 1:2], AF.Sqrt, bias=eps_t)
                rstd = sp.tile([128, 1], F32, tag="rs")
                nc.vector.reciprocal(rstd, std)
                y = yp.tile([128, C], BF16, tag="y")
                nc.vector.tensor_scalar(
                    out=y, in0=xt, scalar1=mv[:, 0:1], scalar2=rstd,
                    op0=OP.subtract, op1=OP.mult)
                tpb = tpp.tile([128, CCH, 128], BF16, tag=tag, name="tpb")
                for cc in range(CCH):
                    nc.tensor.transpose(
                        tpb[:, cc, :], y[:, cc * 128:(cc + 1) * 128], ident)
                nc.vector.tensor_copy(
                    out=yT[:, :, t * 128:(t + 1) * 128], in_=tpb)

        # A2 scratch
        a2s = top.enter_context(tc.tile_pool(name="a2s", bufs=2))
        # B scratch
        ep = top.enter_context(tc.tile_pool(name="ep", bufs=3))
        rp = top.enter_context(tc.tile_pool(name="rp", bufs=1))
        bcsp = top.enter_context(tc.tile_pool(name="bcsp", bufs=2))
        tb = top.enter_context(tc.tile_pool(name="tb", bufs=2))
        # C scratch
        op_ = top.enter_context(tc.tile_pool(name="op", bufs=2))

        def A2(b, pp, sqp, bcp):
            for wi, (wn, o_big) in enumerate((("q", qT), ("k", kT))):
                wsb = w_sb[wn]
                for dc in range(CCH):
                    for t2 in range(2):
                        ps = pp.tile([128, 512], F32, tag="ps")
                        for cc in range(CCH):
                            nc.tensor.matmul(
                                ps, wsb[:, cc, dc * 128:(dc + 1) * 128],
                                yT[:, cc, t2 * 512:(t2 + 1) * 512],
                                start=(cc == 0), stop=(cc == CCH - 1))
                        sq = a2s.tile([128, 512], BF16, tag="sq")
                        nc.scalar.activation(sq, ps, AF.Square)
                        qraw = a2s.tile([128, 512], BF16, tag="qraw")
                        nc.vector.tensor_copy(out=qraw, in_=ps)
                        ssq = sqp.tile([2, 512], F32, tag="ssq")
                        nc.tensor.matmul(ssq, e2, sq, start=True, stop=True)
                        stdt = a2s.tile([2, 512], BF16, tag="stdt")
                        if wi == 0:
                            # 0.125/sqrt(ssq/64+eps) = 1/sqrt(ssq+64eps)
                            nc.scalar.activation(
                                stdt, ssq, AF.Sqrt, bias=eps64_t[0:2, :])
                        else:
                            nc.scalar.activation(
                                stdt, ssq, AF.Sqrt, bias=eps_t[0:2, :],
                                scale=1.0 / 64.0)
                        rst = a2s.tile([2, 512], BF16, tag="rst")
                        with nc.allow_low_precision(reason="bf16 rstd"):
                            nc.vector.reciprocal(rst, stdt)
                        bc = bcp.tile([128, 512], F32, tag="bc")
                        nc.tensor.matmul(bc, b2, rst, start=True, stop=True)
                        nc.vector.tensor_mul(
                            o_big[:, dc, t2 * 512:(t2 + 1) * 512], qraw, bc)
            # v projection + ones column
            wsb = w_sb["v"]
            for tt in range(NB):
                for d2 in range(2):
                    ps = pp.tile([128, 512], F32, tag="ps")
                    for cc in range(CCH):
                        nc.tensor.matmul(
                            ps, yT[:, cc, tt * 128:(tt + 1) * 128],
                            wsb[:, cc, d2 * 512:(d2 + 1) * 512],
                            start=(cc == 0), stop=(cc == CCH - 1))
                    nc.vector.tensor_copy(
                        out=vS[:, tt, d2 * 8:(d2 + 1) * 8, 0:64],
                        in_=ps.rearrange("p (h e) -> p h e", e=64))
                nc.vector.tensor_copy(
                    out=vS[:, tt, :, 64:65],
                    in_=cones[:, 0:H].rearrange("p (h e) -> p h e", e=1))

        def Bphase(b, swin, oop, nbc, cpp, cunits, first):
            AOc = AO[b % 2]
            def emit_S(hp, qc):
                sw = swin.tile([128, 1024], F32, tag="sw", name="sw")
                nc.tensor.matmul(
                    sw[:, 0:512],
                    kT[0:64, hp, emit_S.kc * 128:(emit_S.kc + 1) * 128],
                    qT[0:64, hp, qc * 512:(qc + 1) * 512],
                    start=True, stop=True)
                nc.tensor.matmul(
                    sw[:, 512:1024],
                    kT[64:128, hp, emit_S.kc * 128:(emit_S.kc + 1) * 128],
                    qT[64:128, hp, qc * 512:(qc + 1) * 512],
                    start=True, stop=True)
                return sw

            for qc in range(2):
                for hp in range(H // 2):
                    cu = cunits.pop(0) if (qc == 1 or not first) and cunits \
                        else None
                    if cu is not None:
                        cu()
                    oo = oop.tile([65, 2, 512], F32, tag="oo")
                    # software-pipelined by one kc stage: S(kc+1) is emitted
                    # before exp(kc)/O(kc) so PE fills the exp window and
                    # ScalarE never starves.
                    emit_S.kc = 0
                    sw_prev = emit_S(hp, qc)
                    for kc in range(NB):
                        if kc + 1 < NB:
                            emit_S.kc = kc + 1
                            sw_next = emit_S(hp, qc)
                        ew = ep.tile([128, 1024], BF16, tag="ew")
                        nc.scalar.activation(ew, sw_prev, AF.Exp)
                        nc.tensor.matmul(
                            oo[:, 0, :], vS[:, kc, 2 * hp, :], ew[:, 0:512],
                            start=(kc == 0), stop=(kc == NB - 1))
                        nc.tensor.matmul(
                            oo[:, 1, :], vS[:, kc, 2 * hp + 1, :],
                            ew[:, 512:1024],
                            start=(kc == 0), stop=(kc == NB - 1))
                        if kc + 1 < NB:
                            sw_prev = sw_next
                    r2 = rp.tile([128, 2, 512], BF16, tag="r2")
                    with nc.allow_low_precision(reason="bf16 recip"):
                        nc.vector.reciprocal(r2[64:65, :, :], oo[64:65, :, :])
                    osb = bcsp.tile([64, 2, 512], BF16, tag="osb")
                    nc.vector.tensor_copy(out=osb, in_=oo[0:64, :, :])
                    bc0 = nbc.tile([64, 512], F32, tag="nbc")
                    nc.tensor.matmul(
                        bc0, cones[64:65, :], r2[64:65, 0, :],
                        start=True, stop=True, tile_position=(64, 0))
                    bc1 = nbc.tile([64, 512], F32, tag="nbc")
                    nc.tensor.matmul(
                        bc1, cones[64:65, :], r2[64:65, 1, :],
                        start=True, stop=True, tile_position=(64, 0))
                    nc.vector.tensor_mul(
                        AOc[0:64, hp, qc * 512:(qc + 1) * 512],
                        osb[:, 0, :], bc0)
                    t2b = tb.tile([64, 512], BF16, tag="t2b")
                    nc.vector.tensor_mul(t2b, osb[:, 1, :], bc1)
                    nc.sync.dma_start(
                        out=AOc[64:128, hp, qc * 512:(qc + 1) * 512], in_=t2b)

        def C_unit(b, pp, tt, d2, tag="ps"):
            AOc = AO[b % 2]
            wsb = w_sb["p"]
            ps = pp.tile([128, 512], F32, tag=tag, name="cps")
            for cc in range(CCH):
                nc.tensor.matmul(
                    ps, AOc[:, cc, tt * 128:(tt + 1) * 128],
                    wsb[:, cc, d2 * 512:(d2 + 1) * 512],
                    start=(cc == 0), stop=(cc == CCH - 1))
            o_sb = op_.tile([128, 512], F32, tag="osb")
            nc.vector.tensor_add(
                o_sb, ps, bpb[:, d2 * 512:(d2 + 1) * 512])
            nc.sync.dma_start(
                out=out_d[b * N + tt * 128:b * N + (tt + 1) * 128,
                          d2 * 512:(d2 + 1) * 512],
                in_=o_sb)

        # ---- pipelined schedule ----
        # A1(b)'s LN work (DMA + DVE) has no PSUM deps, so it executes during
        # B(b-1); only its transposes wait for the scope1 banks.
        for b in range(BL):
            with ExitStack() as ph:
                pp = ph.enter_context(
                    tc.tile_pool(name="pp", bufs=4, space="PSUM"))
                sqp = ph.enter_context(
                    tc.tile_pool(name="sqp", bufs=2, space="PSUM"))
                bcp = ph.enter_context(
                    tc.tile_pool(name="bcp", bufs=2, space="PSUM"))
                A1(b, pp)
                if b == 0:
                    for nm in ("q", "k", "v", "p"):
                        load_weight(nm)
                    nc.sync.dma_start(out=bpb, in_=bpb_d)
                A2(b, pp, sqp, bcp)
            with ExitStack() as ph:
                swin = ph.enter_context(
                    tc.tile_pool(name="swin", bufs=2, space="PSUM"))
                oop = ph.enter_context(
                    tc.tile_pool(name="oop", bufs=1, space="PSUM"))
                nbc = ph.enter_context(
                    tc.tile_pool(name="nbc", bufs=1, space="PSUM"))
                cpp = ph.enter_context(
                    tc.tile_pool(name="cpp", bufs=1, space="PSUM"))
                # C units of the previous batch (all ready) and of this
                # batch's qc=0 half (ready once qc=0 completes) fill the PE
                # idle inside the ACT-bound B window.
                cunits = []
                if b > 0:
                    cunits += [
                        (lambda tt=tt, d2=d2: C_unit(b - 1, cpp, tt, d2,
                                                     tag="cps"))
                        for tt in range(4, NB) for d2 in range(2)]
                cunits += [
                    (lambda tt=tt, d2=d2: C_unit(b, cpp, tt, d2, tag="cps"))
                    for tt in range(0, 4) for d2 in range(2)]
                Bphase(b, swin, oop, nbc, cpp, cunits, first=(b == 0))
        with ExitStack() as ph:
            ppc = ph.enter_context(
                tc.tile_pool(name="ppc", bufs=3, space="PSUM"))
            for tt in range(4, NB):
                for d2 in range(2):
                    C_unit(BL - 1, ppc, tt, d2)

    nc.compile()
    return nc


def _get_nc():
    if "nc" not in _cache:
        _cache["nc"] = _build()
    return _cache["nc"]


def _host_inputs(Wq, Wk, Wv, Wp, bp):
    """Shared (core-independent) derived weight tensors."""
    import ml_dtypes
    BF = ml_dtypes.bfloat16

    def center(Wm):
        Wh = np.asarray(Wm, np.float32).reshape(H, Dh, C)
        return (Wh - Wh.mean(axis=1, keepdims=True)).reshape(C, C)

    e2 = np.zeros((128, 2), np.float32)
    e2[0:64, 0] = 1.0
    e2[64:128, 1] = 1.0
    b2 = np.zeros((2, 128), np.float32)
    b2[0, 0:64] = 1.0
    b2[1, 64:128] = 1.0
    eps = np.zeros((128, 2), np.float32)
    eps[:, 0] = EPS
    eps[:, 1] = 64.0 * EPS
    bpb = np.broadcast_to(np.asarray(bp, np.float32).reshape(1, C),
                          (128, C))
    return {
        "c_e2": e2.astype(BF),
        "c_b2": b2.astype(BF),
        "c_ones": np.ones((128, 64), BF),
        "c_ident": np.eye(128, dtype=np.float32).astype(BF),
        "c_eps": eps,
        "bpb": np.ascontiguousarray(bpb).astype(BF),
        "wqt": np.ascontiguousarray(center(Wq).T).astype(BF),
        "wkt": np.ascontiguousarray(center(Wk).T).astype(BF),
        "wvt": np.ascontiguousarray(np.asarray(Wv, np.float32).T).astype(BF),
        "wpt": np.ascontiguousarray(np.asarray(Wp, np.float32).T).astype(BF),
    }


def kernel(x, Wq, Wk, Wv, Wp, bp):
    from concourse.bass_utils import run_bass_kernel_spmd

    nc = _get_nc()
    shared = _host_inputs(Wq, Wk, Wv, Wp, bp)
    x = np.asarray(x, dtype=np.float32)
    in_maps = [
        dict(shared, x=np.ascontiguousarray(x[c * BL:(c + 1) * BL].reshape(T, C)))
        for c in range(NCORES)
    ]
    res = run_bass_kernel_spmd(nc, in_maps, core_ids=list(range(NCORES)))
    out = np.stack([res.results[c]["out"].reshape(BL, N, C)
                    for c in range(NCORES)])
    return out.reshape(B, N, C).astype(np.float32)


# revision 44
# speedup vs baseline: 1.2055x; 1.0329x over previous
"""Trainium2 Bass kernel for nn_Attention (B=16, N=1024, C=1024, H=16, pre-LN +
q/k post-LN attention block), data-parallel over 8 NeuronCores (2 batches/core).

Per core (batch shard [2, 1024, 1024]), software-pipelined across the two
batches with persistent SBUF buffers and bf16 activations/weights (fp32 PSUM
accumulation and LN statistics):
  A1: y = LN(x) over C (bn_stats fp32); y cast bf16; 8 PE bf16-transposes per
      token tile packed into ONE PSUM bank, evacuated with a single DVE copy.
  A2: qT/kT = W'c @ yT (weights host-pre-centered per head so the post-LN mean
      subtraction folds in; the 1/8 attention scale folds into q's rstd eps);
      per-head rstd via ACT Square + ones-matmul partition reduction + ACT
      sqrt + DVE bf16 reciprocal, broadcast back across partitions with a PE
      matmul; q_raw staged to SBUF so the normalize mul reads only one PSUM
      operand (walrus allows at most one PSUM input per DVE op); v in
      [tok, d] layout with a ones column (softmax denominator for free).
  B:  per head-pair / query-chunk: S^T tiles for both heads land in one
      2-bank PSUM window, ONE exp [128,1024] on ScalarE (the gate of this
      phase; ~92% ACT occupancy), O_aug^T = [V|1]^T E accumulated on PE
      (row 64 = denominator); normalize via one wide DVE reciprocal + one
      wide PSUM->SBUF staging copy + per-head PE broadcasts + DVE muls
      (head1 shifted into partitions 64-127 by a small SBUF-SBUF DMA).
  C:  out = AO^T^T @ Wp^T; bias added during the PSUM evacuation (DVE tensor
      add against a host-broadcast bias tile), fp32 out. C units are
      interleaved INTO the ACT-bound B windows (C(b-1) second half and C(b)
      first half), so only half of the last batch's projection runs serially
      at the end.

Weights are DMA'd once and stay resident in SBUF (bf16, 8 MB). PSUM is the
scarce resource (8 banks): scope1 = {4x matmul/transpose staging, 2x ssq,
2x rstd-broadcast}, scope2 = {2x2 exp windows, 2 O-accumulators, 1 bc,
1 interleaved-C}. Engine budget per core (cost model): PE ~496us busy,
ACT ~349us, DVE ~324us, span ~603us.

Note: gpsimd.partition_broadcast is NOT used — on real TRN2 it only works
for a physical-partition-0 source and partition-0-based destination
(sim/CoreSim does not model this; verified by hardware micro-test).
"""

import numpy as np

B, N, C, H, Dh = 16, 1024, 1024, 16, 64
NCORES = 8
BL = B // NCORES          # batches per core
T = BL * N                # tokens per core
CCH = C // 128            # contraction chunks
NB = N // 128             # token tiles per batch
EPS = 1e-6

_cache: dict = {}


def _build():
    from contextlib import ExitStack

    import concourse.bacc as bacc
    import concourse.mybir as mybir
    import concourse.tile as tile

    F32 = mybir.dt.float32
    BF16 = mybir.dt.bfloat16
    AF = mybir.ActivationFunctionType
    OP = mybir.AluOpType

    nc = bacc.Bacc("TRN2", target_bir_lowering=False, debug=False,
                   num_devices=NCORES)

    x_d = nc.dram_tensor("x", [T, C], F32, kind="ExternalInput").ap()
    wqt_d = nc.dram_tensor("wqt", [C, C], BF16, kind="ExternalInput").ap()
    wkt_d = nc.dram_tensor("wkt", [C, C], BF16, kind="ExternalInput").ap()
    wvt_d = nc.dram_tensor("wvt", [C, C], BF16, kind="ExternalInput").ap()
    wpt_d = nc.dram_tensor("wpt", [C, C], BF16, kind="ExternalInput").ap()
    bpb_d = nc.dram_tensor("bpb", [128, C], BF16, kind="ExternalInput").ap()
    ce2_d = nc.dram_tensor("c_e2", [128, 2], BF16, kind="ExternalInput").ap()
    cident_d = nc.dram_tensor("c_ident", [128, 128], BF16,
                              kind="ExternalInput").ap()
    cones_d = nc.dram_tensor("c_ones", [128, 64], BF16,
                             kind="ExternalInput").ap()
    cb2_d = nc.dram_tensor("c_b2", [2, 128], BF16, kind="ExternalInput").ap()
    ceps_d = nc.dram_tensor("c_eps", [128, 2], F32, kind="ExternalInput").ap()
    out_d = nc.dram_tensor("out", [T, C], F32, kind="ExternalOutput").ap()

    with tile.TileContext(nc) as tc, ExitStack() as top:
        # ---- persistent SBUF ----
        const = top.enter_context(tc.tile_pool(name="const", bufs=1))
        ident = const.tile([128, 128], BF16)
        nc.sync.dma_start(out=ident, in_=cident_d)
        e2 = const.tile([128, 2], BF16)
        nc.sync.dma_start(out=e2, in_=ce2_d)
        b2 = const.tile([2, 128], BF16)
        nc.sync.dma_start(out=b2, in_=cb2_d)
        cones = const.tile([128, 64], BF16)
        nc.sync.dma_start(out=cones, in_=cones_d)
        ceps = const.tile([128, 2], F32)
        nc.sync.dma_start(out=ceps, in_=ceps_d)
        eps_t = ceps[:, 0:1]
        eps64_t = ceps[:, 1:2]
        bpb = const.tile([128, C], BF16)

        wpool = top.enter_context(tc.tile_pool(name="w", bufs=1))
        w_sb = {}
        w_dram = {"q": wqt_d, "k": wkt_d, "v": wvt_d, "p": wpt_d}
        for nm in ("q", "k", "v", "p"):
            w_sb[nm] = wpool.tile([128, CCH, C], BF16, name=f"w{nm}")

        def load_weight(nm):
            nc.sync.dma_start(
                out=w_sb[nm],
                in_=w_dram[nm].rearrange("(cc p) d -> p cc d", p=128))

        big = top.enter_context(tc.tile_pool(name="big", bufs=1))
        yT = big.tile([128, CCH, N], BF16, name="yT")
        qT = big.tile([128, CCH, N], BF16, name="qT")
        kT = big.tile([128, CCH, N], BF16, name="kT")
        vS = big.tile([128, NB, H, Dh + 1], BF16, name="vS")
        AO = [big.tile([128, CCH, N], BF16, name=f"AO{i}") for i in range(2)]

        # A1 SBUF scratch (persistent pools, rotating bufs)
        xp = top.enter_context(tc.tile_pool(name="xp", bufs=2))
        yp = top.enter_context(tc.tile_pool(name="yp", bufs=4))
        sp = top.enter_context(tc.tile_pool(name="sp", bufs=3))

        def A1(b, tpp, tag="ps"):
            for t in range(NB):
                r0 = b * N + t * 128
                xt = xp.tile([128, C], F32, tag="xt")
                nc.sync.dma_start(out=xt, in_=x_d[r0:r0 + 128, :])
                stats = sp.tile([128, 2, nc.vector.BN_STATS_DIM], F32,
                                tag="st")
                xg = xt.rearrange("p (s f) -> p s f", s=2)
                for s in range(2):
                    nc.vector.bn_stats(out=stats[:, s, :], in_=xg[:, s, :])
                mv = sp.tile([128, nc.vector.BN_AGGR_DIM], F32, tag="mv")
                nc.vector.bn_aggr(out=mv, in_=stats)
                std = sp.tile([128, 1], F32, tag="sd")
                nc.scalar.activation(std, mv[:, 1:2], AF.Sqrt, bias=eps_t)
                rstd = sp.tile([128, 1], F32, tag="rs")
                nc.vector.reciprocal(rstd, std)
                y = yp.tile([128, C], BF16, tag="y")
                nc.vector.tensor_scalar(
                    out=y, in0=xt, scalar1=mv[:, 0:1], scalar2=rstd,
                    op0=OP.subtract, op1=OP.mult)
                tpb = tpp.tile([128, CCH, 128], BF16, tag=tag, name="tpb")
                for cc in range(CCH):
                    nc.tensor.transpose(
                        tpb[:, cc, :], y[:, cc * 128:(cc + 1) * 128], ident)
                nc.vector.tensor_copy(
                    out=yT[:, :, t * 128:(t + 1) * 128], in_=tpb)

        # A2 scratch
        a2s = top.enter_context(tc.tile_pool(name="a2s", bufs=2))
        # B scratch
        ep = top.enter_context(tc.tile_pool(name="ep", bufs=3))
        rp = top.enter_context(tc.tile_pool(name="rp", bufs=1))
        bcsp = top.enter_context(tc.tile_pool(name="bcsp", bufs=2))
        tb = top.enter_context(tc.tile_pool(name="tb", bufs=2))
        # C scratch
        op_ = top.enter_context(tc.tile_pool(name="op", bufs=2))

        def A2(b, pp, sqp, bcp):
            for wi, (wn, o_big) in enumerate((("q", qT), ("k", kT))):
                wsb = w_sb[wn]
                for dc in range(CCH):
                    for t2 in range(2):
                        ps = pp.tile([128, 512], F32, tag="ps")
                        for cc in range(CCH):
                            nc.tensor.matmul(
                                ps, wsb[:, cc, dc * 128:(dc + 1) * 128],
                                yT[:, cc, t2 * 512:(t2 + 1) * 512],
                                start=(cc == 0), stop=(cc == CCH - 1))
                        sq = a2s.tile([128, 512], BF16, tag="sq")
                        nc.scalar.activation(sq, ps, AF.Square)
                        qraw = a2s.tile([128, 512], BF16, tag="qraw")
                        nc.vector.tensor_copy(out=qraw, in_=ps)
                        ssq = sqp.tile([2, 512], F32, tag="ssq")
                        nc.tensor.matmul(ssq, e2, sq, start=True, stop=True)
                        stdt = a2s.tile([2, 512], BF16, tag="stdt")
                        if wi == 0:
                            # 0.125/sqrt(ssq/64+eps) = 1/sqrt(ssq+64eps)
                            nc.scalar.activation(
                                stdt, ssq, AF.Sqrt, bias=eps64_t[0:2, :])
                        else:
                            nc.scalar.activation(
                                stdt, ssq, AF.Sqrt, bias=eps_t[0:2, :],
                                scale=1.0 / 64.0)
                        rst = a2s.tile([2, 512], BF16, tag="rst")
                        with nc.allow_low_precision(reason="bf16 rstd"):
                            nc.vector.reciprocal(rst, stdt)
                        bc = bcp.tile([128, 512], F32, tag="bc")
                        nc.tensor.matmul(bc, b2, rst, start=True, stop=True)
                        nc.vector.tensor_mul(
                            o_big[:, dc, t2 * 512:(t2 + 1) * 512], qraw, bc)
            # v projection + ones column
            wsb = w_sb["v"]
            for tt in range(NB):
                for d2 in range(2):
                    ps = pp.tile([128, 512], F32, tag="ps")
                    for cc in range(CCH):
                        nc.tensor.matmul(
                            ps, yT[:, cc, tt * 128:(tt + 1) * 128],
                            wsb[:, cc, d2 * 512:(d2 + 1) * 512],
                            start=(cc == 0), stop=(cc == CCH - 1))
                    nc.vector.tensor_copy(
                        out=vS[:, tt, d2 * 8:(d2 + 1) * 8, 0:64],
                        in_=ps.rearrange("p (h e) -> p h e", e=64))
                nc.vector.tensor_copy(
                    out=vS[:, tt, :, 64:65],
                    in_=cones[:, 0:H].rearrange("p (h e) -> p h e", e=1))

        def Bphase(b, swin, oop, nbc, cpp, cunits, first):
            AOc = AO[b % 2]
            def emit_S(hp, qc):
                sw = swin.tile([128, 1024], F32, tag="sw", name="sw")
                nc.tensor.matmul(
                    sw[:, 0:512],
                    kT[0:64, hp, emit_S.kc * 128:(emit_S.kc + 1) * 128],
                    qT[0:64, hp, qc * 512:(qc + 1) * 512],
                    start=True, stop=True)
                nc.tensor.matmul(
                    sw[:, 512:1024],
                    kT[64:128, hp, emit_S.kc * 128:(emit_S.kc + 1) * 128],
                    qT[64:128, hp, qc * 512:(qc + 1) * 512],
                    start=True, stop=True)
                return sw

            for qc in range(2):
                for hp in range(H // 2):
                    cu = cunits.pop(0) if (qc == 1 or not first) and cunits \
                        else None
                    oo = oop.tile([65, 2, 512], F32, tag="oo")
                    # software-pipelined by one kc stage: S(kc+1) is emitted
                    # before exp(kc)/O(kc) so PE fills the exp window and
                    # ScalarE never starves.
                    emit_S.kc = 0
                    sw_prev = emit_S(hp, qc)
                    for kc in range(NB):
                        if kc + 1 < NB:
                            emit_S.kc = kc + 1
                            sw_next = emit_S(hp, qc)
                        ew = ep.tile([128, 1024], BF16, tag="ew")
                        nc.scalar.activation(ew, sw_prev, AF.Exp)
                        nc.tensor.matmul(
                            oo[:, 0, :], vS[:, kc, 2 * hp, :], ew[:, 0:512],
                            start=(kc == 0), stop=(kc == NB - 1))
                        nc.tensor.matmul(
                            oo[:, 1, :], vS[:, kc, 2 * hp + 1, :],
                            ew[:, 512:1024],
                            start=(kc == 0), stop=(kc == NB - 1))
                        if kc + 1 < NB:
                            sw_prev = sw_next
                    if cu is not None:
                        cu()
                    r2 = rp.tile([128, 2, 512], BF16, tag="r2")
                    with nc.allow_low_precision(reason="bf16 recip"):
                        nc.vector.reciprocal(r2[64:65, :, :], oo[64:65, :, :])
                    osb = bcsp.tile([64, 2, 512], BF16, tag="osb")
                    nc.vector.tensor_copy(out=osb, in_=oo[0:64, :, :])
                    bc0 = nbc.tile([64, 512], F32, tag="nbc")
                    nc.tensor.matmul(
                        bc0, cones[64:65, :], r2[64:65, 0, :],
                        start=True, stop=True, tile_position=(64, 0))
                    bc1 = nbc.tile([64, 512], F32, tag="nbc")
                    nc.tensor.matmul(
                        bc1, cones[64:65, :], r2[64:65, 1, :],
                        start=True, stop=True, tile_position=(64, 0))
                    nc.vector.tensor_mul(
                        AOc[0:64, hp, qc * 512:(qc + 1) * 512],
                        osb[:, 0, :], bc0)
                    t2b = tb.tile([64, 512], BF16, tag="t2b")
                    nc.vector.tensor_mul(t2b, osb[:, 1, :], bc1)
                    nc.sync.dma_start(
                        out=AOc[64:128, hp, qc * 512:(qc + 1) * 512], in_=t2b)

        def C_unit(b, pp, tt, d2, tag="ps"):
            AOc = AO[b % 2]
            wsb = w_sb["p"]
            ps = pp.tile([128, 512], F32, tag=tag, name="cps")
            for cc in range(CCH):
                nc.tensor.matmul(
                    ps, AOc[:, cc, tt * 128:(tt + 1) * 128],
                    wsb[:, cc, d2 * 512:(d2 + 1) * 512],
                    start=(cc == 0), stop=(cc == CCH - 1))
            o_sb = op_.tile([128, 512], F32, tag="osb")
            nc.vector.tensor_add(
                o_sb, ps, bpb[:, d2 * 512:(d2 + 1) * 512])
            nc.sync.dma_start(
                out=out_d[b * N + tt * 128:b * N + (tt + 1) * 128,
                          d2 * 512:(d2 + 1) * 512],
                in_=o_sb)

        # ---- pipelined schedule ----
        # A1(b)'s LN work (DMA + DVE) has no PSUM deps, so it executes during
        # B(b-1); only its transposes wait for the scope1 banks.
        for b in range(BL):
            with ExitStack() as ph:
                pp = ph.enter_context(
                    tc.tile_pool(name="pp", bufs=4, space="PSUM"))
                sqp = ph.enter_context(
                    tc.tile_pool(name="sqp", bufs=2, space="PSUM"))
                bcp = ph.enter_context(
                    tc.tile_pool(name="bcp", bufs=2, space="PSUM"))
                A1(b, pp)
                if b == 0:
                    for nm in ("q", "k", "v", "p"):
                        load_weight(nm)
                    nc.sync.dma_start(out=bpb, in_=bpb_d)
                A2(b, pp, sqp, bcp)
            with ExitStack() as ph:
                swin = ph.enter_context(
                    tc.tile_pool(name="swin", bufs=2, space="PSUM"))
                oop = ph.enter_context(
                    tc.tile_pool(name="oop", bufs=1, space="PSUM"))
                nbc = ph.enter_context(
                    tc.tile_pool(name="nbc", bufs=1, space="PSUM"))
                cpp = ph.enter_context(
                    tc.tile_pool(name="cpp", bufs=1, space="PSUM"))
                # C units of the previous batch (all ready) and of this
                # batch's qc=0 half (ready once qc=0 completes) fill the PE
                # idle inside the ACT-bound B window.
                cunits = []
                if b > 0:
                    cunits += [
                        (lambda tt=tt, d2=d2: C_unit(b - 1, cpp, tt, d2,
                                                     tag="cps"))
                        for tt in range(4, NB) for d2 in range(2)]
                cunits += [
                    (lambda tt=tt, d2=d2: C_unit(b, cpp, tt, d2, tag="cps"))
                    for tt in range(0, 4) for d2 in range(2)]
                Bphase(b, swin, oop, nbc, cpp, cunits, first=(b == 0))
        with ExitStack() as ph:
            ppc = ph.enter_context(
                tc.tile_pool(name="ppc", bufs=3, space="PSUM"))
            for tt in range(4, NB):
                for d2 in range(2):
                    C_unit(BL - 1, ppc, tt, d2)

    nc.compile()
    return nc


def _get_nc():
    if "nc" not in _cache:
        _cache["nc"] = _build()
    return _cache["nc"]


def _host_inputs(Wq, Wk, Wv, Wp, bp):
    """Shared (core-independent) derived weight tensors."""
    import ml_dtypes
    BF = ml_dtypes.bfloat16

    def center(Wm):
        Wh = np.asarray(Wm, np.float32).reshape(H, Dh, C)
        return (Wh - Wh.mean(axis=1, keepdims=True)).reshape(C, C)

    e2 = np.zeros((128, 2), np.float32)
    e2[0:64, 0] = 1.0
    e2[64:128, 1] = 1.0
    b2 = np.zeros((2, 128), np.float32)
    b2[0, 0:64] = 1.0
    b2[1, 64:128] = 1.0
    eps = np.zeros((128, 2), np.float32)
    eps[:, 0] = EPS
    eps[:, 1] = 64.0 * EPS
    bpb = np.broadcast_to(np.asarray(bp, np.float32).reshape(1, C),
                          (128, C))
    return {
        "c_e2": e2.astype(BF),
        "c_b2": b2.astype(BF),
        "c_ones": np.ones((128, 64), BF),
        "c_ident": np.eye(128, dtype=np.float32).astype(BF),
        "c_eps": eps,
        "bpb": np.ascontiguousarray(bpb).astype(BF),
        "wqt": np.ascontiguousarray(center(Wq).T).astype(BF),
        "wkt": np.ascontiguousarray(center(Wk).T).astype(BF),
        "wvt": np.ascontiguousarray(np.asarray(Wv, np.float32).T).astype(BF),
        "wpt": np.ascontiguousarray(np.asarray(Wp, np.float32).T).astype(BF),
    }


def kernel(x, Wq, Wk, Wv, Wp, bp):
    from concourse.bass_utils import run_bass_kernel_spmd

    nc = _get_nc()
    shared = _host_inputs(Wq, Wk, Wv, Wp, bp)
    x = np.asarray(x, dtype=np.float32)
    in_maps = [
        dict(shared, x=np.ascontiguousarray(x[c * BL:(c + 1) * BL].reshape(T, C)))
        for c in range(NCORES)
    ]
    res = run_bass_kernel_spmd(nc, in_maps, core_ids=list(range(NCORES)))
    out = np.stack([res.results[c]["out"].reshape(BL, N, C)
                    for c in range(NCORES)])
    return out.reshape(B, N, C).astype(np.float32)


# revision 48
# speedup vs baseline: 1.3388x; 1.1106x over previous
"""Trainium2 Bass kernel for nn_Attention (B=16, N=1024, C=1024, H=16, pre-LN +
q/k post-LN attention block), data-parallel over 8 NeuronCores (2 batches/core).

Per core (batch shard [2, 1024, 1024]), software-pipelined across the two
batches with persistent SBUF buffers and bf16 activations/weights (fp32 PSUM
accumulation and LN statistics):
  A1: y = LN(x) over C (bn_stats fp32); y cast bf16; 8 PE bf16-transposes per
      token tile packed into ONE PSUM bank, evacuated with a single DVE copy.
  A2: qT/kT = W'c @ yT (weights host-pre-centered per head so the post-LN mean
      subtraction folds in; the 1/8 attention scale folds into q's rstd eps);
      per-head rstd via ACT Square + ones-matmul partition reduction + ACT
      sqrt + DVE bf16 reciprocal, broadcast back across partitions with a PE
      matmul; q_raw staged to SBUF so the normalize mul reads only one PSUM
      operand (walrus allows at most one PSUM input per DVE op); v in
      [tok, d] layout with a ones column (softmax denominator for free).
  B:  per head-pair / query-chunk: S^T tiles for both heads land in one
      2-bank PSUM window, ONE exp [128,1024] on ScalarE (the gate of this
      phase; ~92% ACT occupancy), O_aug^T = [V|1]^T E accumulated on PE
      (row 64 = denominator); normalize via one wide DVE reciprocal + one
      wide PSUM->SBUF staging copy + per-head PE broadcasts + DVE muls
      (head1 shifted into partitions 64-127 by a small SBUF-SBUF DMA).
  C:  out = AO^T^T @ Wp^T; bias added during the PSUM evacuation (DVE tensor
      add against a host-broadcast bias tile), fp32 out. C units are
      interleaved INTO the ACT-bound B windows (C(b-1) second half and C(b)
      first half), so only half of the last batch's projection runs serially
      at the end.

Weights are DMA'd once and stay resident in SBUF (bf16, 8 MB). PSUM is the
scarce resource (8 banks): scope1 = {4x matmul/transpose staging, 2x ssq,
2x rstd-broadcast}, scope2 = {2x2 exp windows, 2 O-accumulators, 1 bc,
1 interleaved-C}. Engine budget per core (cost model): PE ~496us busy,
ACT ~349us, DVE ~324us, span ~603us.

Note: gpsimd.partition_broadcast is NOT used — on real TRN2 it only works
for a physical-partition-0 source and partition-0-based destination
(sim/CoreSim does not model this; verified by hardware micro-test).
"""

import numpy as np

B, N, C, H, Dh = 16, 1024, 1024, 16, 64
NCORES = 8
BL = B // NCORES          # batches per core
T = BL * N                # tokens per core
CCH = C // 128            # contraction chunks
NB = N // 128             # token tiles per batch
EPS = 1e-6

_cache: dict = {}


def _build():
    from contextlib import ExitStack

    import concourse.bacc as bacc
    import concourse.mybir as mybir
    import concourse.tile as tile

    F32 = mybir.dt.float32
    BF16 = mybir.dt.bfloat16
    AF = mybir.ActivationFunctionType
    OP = mybir.AluOpType

    nc = bacc.Bacc("TRN2", target_bir_lowering=False, debug=False,
                   num_devices=NCORES)

    x_d = nc.dram_tensor("x", [T, C], F32, kind="ExternalInput").ap()
    wqt_d = nc.dram_tensor("wqt", [C, C], BF16, kind="ExternalInput").ap()
    wkt_d = nc.dram_tensor("wkt", [C, C], BF16, kind="ExternalInput").ap()
    wvt_d = nc.dram_tensor("wvt", [C, C], BF16, kind="ExternalInput").ap()
    wpt_d = nc.dram_tensor("wpt", [C, C], BF16, kind="ExternalInput").ap()
    bpb_d = nc.dram_tensor("bpb", [128, C], BF16, kind="ExternalInput").ap()
    ce2_d = nc.dram_tensor("c_e2", [128, 2], BF16, kind="ExternalInput").ap()
    cident_d = nc.dram_tensor("c_ident", [128, 128], BF16,
                              kind="ExternalInput").ap()
    cones_d = nc.dram_tensor("c_ones", [128, 64], BF16,
                             kind="ExternalInput").ap()
    cb2_d = nc.dram_tensor("c_b2", [2, 128], BF16, kind="ExternalInput").ap()
    ceps_d = nc.dram_tensor("c_eps", [128, 2], F32, kind="ExternalInput").ap()
    out_d = nc.dram_tensor("out", [T, C], F32, kind="ExternalOutput").ap()

    with tile.TileContext(nc) as tc, ExitStack() as top:
        # ---- persistent SBUF ----
        const = top.enter_context(tc.tile_pool(name="const", bufs=1))
        ident = const.tile([128, 128], BF16)
        nc.sync.dma_start(out=ident, in_=cident_d)
        e2 = const.tile([128, 2], BF16)
        nc.sync.dma_start(out=e2, in_=ce2_d)
        b2 = const.tile([2, 128], BF16)
        nc.sync.dma_start(out=b2, in_=cb2_d)
        cones = const.tile([128, 64], BF16)
        nc.sync.dma_start(out=cones, in_=cones_d)
        ceps = const.tile([128, 2], F32)
        nc.sync.dma_start(out=ceps, in_=ceps_d)
        eps_t = ceps[:, 0:1]
        eps64_t = ceps[:, 1:2]
        bpb = const.tile([128, C], BF16)

        wpool = top.enter_context(tc.tile_pool(name="w", bufs=1))
        w_sb = {}
        w_dram = {"q": wqt_d, "k": wkt_d, "v": wvt_d, "p": wpt_d}
        for nm in ("q", "k", "v", "p"):
            w_sb[nm] = wpool.tile([128, CCH, C], BF16, name=f"w{nm}")

        def load_weight(nm):
            nc.sync.dma_start(
                out=w_sb[nm],
                in_=w_dram[nm].rearrange("(cc p) d -> p cc d", p=128))

        big = top.enter_context(tc.tile_pool(name="big", bufs=1))
        yT = big.tile([128, CCH, N], BF16, name="yT")
        qT = big.tile([128, CCH, N], BF16, name="qT")
        kT = big.tile([128, CCH, N], BF16, name="kT")
        vS = big.tile([128, NB, H, Dh + 1], BF16, name="vS")
        AO = [big.tile([128, CCH, N], BF16, name=f"AO{i}") for i in range(2)]

        # A1 SBUF scratch (persistent pools, rotating bufs)
        xp = top.enter_context(tc.tile_pool(name="xp", bufs=2))
        yp = top.enter_context(tc.tile_pool(name="yp", bufs=4))
        sp = top.enter_context(tc.tile_pool(name="sp", bufs=3))

        def A1(b, tpp, tag="ps"):
            for t in range(NB):
                r0 = b * N + t * 128
                xt = xp.tile([128, C], F32, tag="xt")
                nc.sync.dma_start(out=xt, in_=x_d[r0:r0 + 128, :])
                stats = sp.tile([128, 2, nc.vector.BN_STATS_DIM], F32,
                                tag="st")
                xg = xt.rearrange("p (s f) -> p s f", s=2)
                for s in range(2):
                    nc.vector.bn_stats(out=stats[:, s, :], in_=xg[:, s, :])
                mv = sp.tile([128, nc.vector.BN_AGGR_DIM], F32, tag="mv")
                nc.vector.bn_aggr(out=mv, in_=stats)
                std = sp.tile([128, 1], F32, tag="sd")
                nc.scalar.activation(std, mv[:, 1:2], AF.Sqrt, bias=eps_t)
                rstd = sp.tile([128, 1], F32, tag="rs")
                nc.vector.reciprocal(rstd, std)
                y = yp.tile([128, C], BF16, tag="y")
                nc.vector.tensor_scalar(
                    out=y, in0=xt, scalar1=mv[:, 0:1], scalar2=rstd,
                    op0=OP.subtract, op1=OP.mult)
                tpb = tpp.tile([128, CCH, 128], BF16, tag=tag, name="tpb")
                for cc in range(CCH):
                    nc.tensor.transpose(
                        tpb[:, cc, :], y[:, cc * 128:(cc + 1) * 128], ident)
                nc.scalar.copy(
                    out=yT[:, :, t * 128:(t + 1) * 128], in_=tpb)

        # A2 scratch
        a2s = top.enter_context(tc.tile_pool(name="a2s", bufs=2))
        # B scratch
        ep = top.enter_context(tc.tile_pool(name="ep", bufs=3))
        rp = top.enter_context(tc.tile_pool(name="rp", bufs=1))
        bcsp = top.enter_context(tc.tile_pool(name="bcsp", bufs=2))
        tb = top.enter_context(tc.tile_pool(name="tb", bufs=2))
        # C scratch
        op_ = top.enter_context(tc.tile_pool(name="op", bufs=2))

        def A2(b, pp, sqp, bcp):
            for wi, (wn, o_big) in enumerate((("q", qT), ("k", kT))):
                wsb = w_sb[wn]
                for dc in range(CCH):
                    for t2 in range(2):
                        ps = pp.tile([128, 512], F32, tag="ps")
                        for cc in range(CCH):
                            nc.tensor.matmul(
                                ps, wsb[:, cc, dc * 128:(dc + 1) * 128],
                                yT[:, cc, t2 * 512:(t2 + 1) * 512],
                                start=(cc == 0), stop=(cc == CCH - 1))
                        sq = a2s.tile([128, 512], BF16, tag="sq")
                        nc.scalar.activation(sq, ps, AF.Square)
                        qraw = a2s.tile([128, 512], BF16, tag="qraw")
                        nc.scalar.copy(out=qraw, in_=ps)
                        ssq = sqp.tile([2, 512], F32, tag="ssq")
                        nc.tensor.matmul(ssq, e2, sq, start=True, stop=True)
                        stdt = a2s.tile([2, 512], BF16, tag="stdt")
                        if wi == 0:
                            # 0.125/sqrt(ssq/64+eps) = 1/sqrt(ssq+64eps)
                            nc.scalar.activation(
                                stdt, ssq, AF.Sqrt, bias=eps64_t[0:2, :])
                        else:
                            nc.scalar.activation(
                                stdt, ssq, AF.Sqrt, bias=eps_t[0:2, :],
                                scale=1.0 / 64.0)
                        rst = a2s.tile([2, 512], BF16, tag="rst")
                        with nc.allow_low_precision(reason="bf16 rstd"):
                            nc.vector.reciprocal(rst, stdt)
                        bc = bcp.tile([128, 512], F32, tag="bc")
                        nc.tensor.matmul(bc, b2, rst, start=True, stop=True)
                        nc.vector.tensor_mul(
                            o_big[:, dc, t2 * 512:(t2 + 1) * 512], qraw, bc)
            # v projection + ones column
            wsb = w_sb["v"]
            for tt in range(NB):
                for d2 in range(2):
                    ps = pp.tile([128, 512], F32, tag="ps")
                    for cc in range(CCH):
                        nc.tensor.matmul(
                            ps, yT[:, cc, tt * 128:(tt + 1) * 128],
                            wsb[:, cc, d2 * 512:(d2 + 1) * 512],
                            start=(cc == 0), stop=(cc == CCH - 1))
                    nc.scalar.copy(
                        out=vS[:, tt, d2 * 8:(d2 + 1) * 8, 0:64],
                        in_=ps.rearrange("p (h e) -> p h e", e=64))
                nc.vector.tensor_copy(
                    out=vS[:, tt, :, 64:65],
                    in_=cones[:, 0:H].rearrange("p (h e) -> p h e", e=1))

        def Bphase(b, swin, oop, nbc, cpp, cunits, first):
            AOc = AO[b % 2]
            def emit_S(hp, qc):
                sw = swin.tile([128, 1024], F32, tag="sw", name="sw")
                nc.tensor.matmul(
                    sw[:, 0:512],
                    kT[0:64, hp, emit_S.kc * 128:(emit_S.kc + 1) * 128],
                    qT[0:64, hp, qc * 512:(qc + 1) * 512],
                    start=True, stop=True)
                nc.tensor.matmul(
                    sw[:, 512:1024],
                    kT[64:128, hp, emit_S.kc * 128:(emit_S.kc + 1) * 128],
                    qT[64:128, hp, qc * 512:(qc + 1) * 512],
                    start=True, stop=True)
                return sw

            for qc in range(2):
                for hp in range(H // 2):
                    cu = cunits.pop(0) if (qc == 1 or not first) and cunits \
                        else None
                    oo = oop.tile([65, 2, 512], F32, tag="oo")
                    # software-pipelined by one kc stage: S(kc+1) is emitted
                    # before exp(kc)/O(kc) so PE fills the exp window and
                    # ScalarE never starves.
                    emit_S.kc = 0
                    sw_prev = emit_S(hp, qc)
                    for kc in range(NB):
                        if kc + 1 < NB:
                            emit_S.kc = kc + 1
                            sw_next = emit_S(hp, qc)
                        ew = ep.tile([128, 1024], BF16, tag="ew")
                        nc.scalar.activation(ew, sw_prev, AF.Exp)
                        nc.tensor.matmul(
                            oo[:, 0, :], vS[:, kc, 2 * hp, :], ew[:, 0:512],
                            start=(kc == 0), stop=(kc == NB - 1))
                        nc.tensor.matmul(
                            oo[:, 1, :], vS[:, kc, 2 * hp + 1, :],
                            ew[:, 512:1024],
                            start=(kc == 0), stop=(kc == NB - 1))
                        if kc + 1 < NB:
                            sw_prev = sw_next
                    if cu is not None:
                        cu()
                    r2 = rp.tile([128, 2, 512], BF16, tag="r2")
                    with nc.allow_low_precision(reason="bf16 recip"):
                        nc.vector.reciprocal(r2[64:65, :, :], oo[64:65, :, :])
                    osb = bcsp.tile([64, 2, 512], BF16, tag="osb")
                    nc.vector.tensor_copy(out=osb, in_=oo[0:64, :, :])
                    bc0 = nbc.tile([64, 512], F32, tag="nbc")
                    nc.tensor.matmul(
                        bc0, cones[64:65, :], r2[64:65, 0, :],
                        start=True, stop=True, tile_position=(64, 0))
                    bc1 = nbc.tile([64, 512], F32, tag="nbc")
                    nc.tensor.matmul(
                        bc1, cones[64:65, :], r2[64:65, 1, :],
                        start=True, stop=True, tile_position=(64, 0))
                    nc.vector.tensor_mul(
                        AOc[0:64, hp, qc * 512:(qc + 1) * 512],
                        osb[:, 0, :], bc0)
                    t2b = tb.tile([64, 512], BF16, tag="t2b")
                    nc.vector.tensor_mul(t2b, osb[:, 1, :], bc1)
                    nc.sync.dma_start(
                        out=AOc[64:128, hp, qc * 512:(qc + 1) * 512], in_=t2b)

        def C_unit(b, pp, tt, d2, tag="ps"):
            AOc = AO[b % 2]
            wsb = w_sb["p"]
            ps = pp.tile([128, 512], F32, tag=tag, name="cps")
            for cc in range(CCH):
                nc.tensor.matmul(
                    ps, AOc[:, cc, tt * 128:(tt + 1) * 128],
                    wsb[:, cc, d2 * 512:(d2 + 1) * 512],
                    start=(cc == 0), stop=(cc == CCH - 1))
            o_sb = op_.tile([128, 512], F32, tag="osb")
            nc.vector.tensor_add(
                o_sb, ps, bpb[:, d2 * 512:(d2 + 1) * 512])
            nc.sync.dma_start(
                out=out_d[b * N + tt * 128:b * N + (tt + 1) * 128,
                          d2 * 512:(d2 + 1) * 512],
                in_=o_sb)

        # ---- pipelined schedule ----
        # A1(b)'s LN work (DMA + DVE) has no PSUM deps, so it executes during
        # B(b-1); only its transposes wait for the scope1 banks.
        for b in range(BL):
            with ExitStack() as ph:
                pp = ph.enter_context(
                    tc.tile_pool(name="pp", bufs=4, space="PSUM"))
                sqp = ph.enter_context(
                    tc.tile_pool(name="sqp", bufs=2, space="PSUM"))
                bcp = ph.enter_context(
                    tc.tile_pool(name="bcp", bufs=2, space="PSUM"))
                A1(b, pp)
                if b == 0:
                    for nm in ("q", "k", "v", "p"):
                        load_weight(nm)
                    nc.sync.dma_start(out=bpb, in_=bpb_d)
                A2(b, pp, sqp, bcp)
            with ExitStack() as ph:
                swin = ph.enter_context(
                    tc.tile_pool(name="swin", bufs=2, space="PSUM"))
                oop = ph.enter_context(
                    tc.tile_pool(name="oop", bufs=1, space="PSUM"))
                nbc = ph.enter_context(
                    tc.tile_pool(name="nbc", bufs=1, space="PSUM"))
                cpp = ph.enter_context(
                    tc.tile_pool(name="cpp", bufs=1, space="PSUM"))
                # C units of the previous batch (all ready) and of this
                # batch's qc=0 half (ready once qc=0 completes) fill the PE
                # idle inside the ACT-bound B window.
                cunits = []
                if b > 0:
                    cunits += [
                        (lambda tt=tt, d2=d2: C_unit(b - 1, cpp, tt, d2,
                                                     tag="cps"))
                        for tt in range(4, NB) for d2 in range(2)]
                cunits += [
                    (lambda tt=tt, d2=d2: C_unit(b, cpp, tt, d2, tag="cps"))
                    for tt in range(0, 4) for d2 in range(2)]
                Bphase(b, swin, oop, nbc, cpp, cunits, first=(b == 0))
        with ExitStack() as ph:
            ppc = ph.enter_context(
                tc.tile_pool(name="ppc", bufs=3, space="PSUM"))
            for tt in range(4, NB):
                for d2 in range(2):
                    C_unit(BL - 1, ppc, tt, d2)

    nc.compile()
    return nc


def _get_nc():
    if "nc" not in _cache:
        _cache["nc"] = _build()
    return _cache["nc"]


def _host_inputs(Wq, Wk, Wv, Wp, bp):
    """Shared (core-independent) derived weight tensors."""
    import ml_dtypes
    BF = ml_dtypes.bfloat16

    def center(Wm):
        Wh = np.asarray(Wm, np.float32).reshape(H, Dh, C)
        return (Wh - Wh.mean(axis=1, keepdims=True)).reshape(C, C)

    e2 = np.zeros((128, 2), np.float32)
    e2[0:64, 0] = 1.0
    e2[64:128, 1] = 1.0
    b2 = np.zeros((2, 128), np.float32)
    b2[0, 0:64] = 1.0
    b2[1, 64:128] = 1.0
    eps = np.zeros((128, 2), np.float32)
    eps[:, 0] = EPS
    eps[:, 1] = 64.0 * EPS
    bpb = np.broadcast_to(np.asarray(bp, np.float32).reshape(1, C),
                          (128, C))
    return {
        "c_e2": e2.astype(BF),
        "c_b2": b2.astype(BF),
        "c_ones": np.ones((128, 64), BF),
        "c_ident": np.eye(128, dtype=np.float32).astype(BF),
        "c_eps": eps,
        "bpb": np.ascontiguousarray(bpb).astype(BF),
        "wqt": np.ascontiguousarray(center(Wq).T).astype(BF),
        "wkt": np.ascontiguousarray(center(Wk).T).astype(BF),
        "wvt": np.ascontiguousarray(np.asarray(Wv, np.float32).T).astype(BF),
        "wpt": np.ascontiguousarray(np.asarray(Wp, np.float32).T).astype(BF),
    }


def kernel(x, Wq, Wk, Wv, Wp, bp):
    from concourse.bass_utils import run_bass_kernel_spmd

    nc = _get_nc()
    shared = _host_inputs(Wq, Wk, Wv, Wp, bp)
    x = np.asarray(x, dtype=np.float32)
    in_maps = [
        dict(shared, x=np.ascontiguousarray(x[c * BL:(c + 1) * BL].reshape(T, C)))
        for c in range(NCORES)
    ]
    res = run_bass_kernel_spmd(nc, in_maps, core_ids=list(range(NCORES)))
    out = np.stack([res.results[c]["out"].reshape(BL, N, C)
                    for c in range(NCORES)])
    return out.reshape(B, N, C).astype(np.float32)
